# revision 1
# baseline (speedup 1.0000x reference)
"""Bass/Trainium2 kernel for nn_CapsuleLayer (dynamic routing capsule layer).

Reference computation:
    inputs: [B=32, J=2048, I=64], W: [K=32, J=2048, D=32, I=64]
    inputs_hat[b,k,j,d] = sum_i inputs[b,j,i] * W[k,j,d,i]
    3 routing iterations (softmax over K), output = squash(s_2)  [B, K, D]

Sharding: J (input capsules) split 8 ways -> J_loc = 256 per core.
Routing softmax (over K) is fully local; only the per-iteration
s[b,k,d] = sum_j c*hat partial sums need a 128KB AllReduce.

Device layouts (per core):
  x stations : [NPAIR=128, 128, 64]  fp16, block-diag pairs (2 j per station)
  W moving   : [NPAIR=128, 128, 1024] fp16 = [pair, (jp,i), (d,k)]
  hat        : SBUF fp16 [128, 64, 1024] = [(jj,b), group, (d,k)]
  s / outputs: [32, 1024] fp32 = [b, (d,k)]
"""

import os
import sys
import numpy as np

import concourse.bass as bass
import concourse.mybir as mybir
import concourse.tile as tile
from concourse import bacc
from concourse import bass_utils

AF = mybir.ActivationFunctionType
ALU = mybir.AluOpType
F16 = mybir.dt.float16
F32 = mybir.dt.float32

EPS = 1e-07
N_CORES = 8
B = 32          # batch
J = 2048        # input capsules (total)
I = 64          # input capsule dim
K = 32          # output capsules
D = 32          # output capsule dim
JL = J // N_CORES          # 256 local input capsules
NPAIR = JL // 2            # 128 station pairs
NGRP = JL // 4             # 64 groups of 4 j's
GPC = 8                    # groups per chunk in routing passes
NCHUNK = NGRP // GPC       # 16 chunks
DK = D * K                 # 1024


def build_program():
    """Build the SPMD bass program (same program on all 8 cores)."""
    nc = bacc.Bacc("TRN2", target_bir_lowering=False, debug=False,
                   enable_asserts=False, num_devices=N_CORES)

    xs = nc.dram_tensor("xs", [NPAIR, 128, I], F16, kind="ExternalInput").ap()
    wt = nc.dram_tensor("wt", [NPAIR, 128, DK], F16, kind="ExternalInput").ap()
    diag = nc.dram_tensor("diag", [128, B], F16, kind="ExternalInput").ap()
    out_d = nc.dram_tensor("out", [B, DK], F32, kind="ExternalOutput").ap()

    with tile.TileContext(nc) as tc:
        _emit(tc, xs, wt, diag, out_d)
    nc.compile()
    return nc


def _emit(tc, xs, wt, diag, out_d):
    nc = tc.nc
    with (
        tc.tile_pool(name="hat", bufs=1) as hat_pool,
        tc.tile_pool(name="wld", bufs=3) as w_pool,
        tc.tile_pool(name="xld", bufs=2) as x_pool,
        tc.tile_pool(name="big", bufs=2) as big_pool,       # prod/ch chunk tiles
        tc.tile_pool(name="tree", bufs=1) as tree_pool,
        tc.tile_pool(name="smx", bufs=1) as smx_pool,
        tc.tile_pool(name="small", bufs=1) as small_pool,
        tc.tile_pool(name="obc", bufs=1) as obc_pool,
        tc.tile_pool(name="const", bufs=1) as const_pool,
        tc.tile_pool(name="accps", bufs=1, space="PSUM") as acc_psum,
        tc.tile_pool(name="hatps", bufs=3, space="PSUM") as hat_psum,
        tc.tile_pool(name="dram", bufs=6, space="DRAM") as dram_pool,
    ):
        # ---- constants ----
        diag_sb = const_pool.tile([128, B], F16, tag="diag")
        nc.sync.dma_start(diag_sb[:], diag)

        # persistent hat storage: [(jj,b), group, (d,k)] fp16
        hat_sb = hat_pool.tile([128, NGRP, DK], F16, tag="hat")

        # O accumulator (sum of squash outputs over past iterations)
        o_acc = const_pool.tile([B, DK], F32, tag="oacc")

        # ---- Pass A: hat = x @ W; s0 accumulated on DVE from PSUM tiles ----
        s0_acc = const_pool.tile([128, DK], F32, tag="s0acc")
        diag32 = const_pool.tile([128, B], F32, tag="diag32")
        nc.vector.tensor_copy(diag32[:], diag_sb[:])
        for g in range(NGRP):
            xg = x_pool.tile([128, 2, I], F16, tag="x")
            nc.sync.dma_start(xg[:], xs[2 * g: 2 * g + 2].rearrange("q p f -> p q f"))

            ps = hat_psum.tile([128, DK], F32, tag="hatps", name=f"hat_ps{g}")
            for q in (0, 1):            # station pair within group
                wq = w_pool.tile([128, DK], F16, tag="w")
                nc.sync.dma_start(wq[:], wt[2 * g + q])
                for jp in (0, 1):       # j within pair: row-half jp*64
                    jj = q * 2 + jp
                    for h in (0, 1):    # free-dim half
                        nc.tensor.matmul(
                            ps[jj * 32:(jj + 1) * 32, h * 512:(h + 1) * 512],
                            lhsT=xg[jp * 64:(jp + 1) * 64, q, jp * 32:(jp + 1) * 32],
                            rhs=wq[jp * 64:(jp + 1) * 64, h * 512:(h + 1) * 512],
                            start=True, stop=True,
                            tile_position=(jp * 64, jj * 32),
                        )
            # PSUM -> SBUF fp16 (split across ScalarE / VectorE)
            nc.scalar.copy(hat_sb[:, g, 0:512], ps[:, 0:512])
            nc.vector.tensor_copy(hat_sb[:, g, 512:DK], ps[:, 512:DK])
            # s0 accumulation on DVE (PE stays free)
            if g == 0:
                nc.vector.tensor_copy(s0_acc[:], ps[:])
            else:
                nc.vector.tensor_add(s0_acc[:], s0_acc[:], ps[:])
        # jj-sum of s0_acc via 2 diagonal matmuls
        s_ps = acc_psum.tile([128, DK], F32, tag="sacc", name="s0_ps")
        for h in (0, 1):
            nc.tensor.matmul(
                s_ps[0:B, h * 512:(h + 1) * 512],
                lhsT=diag32[:],
                rhs=s0_acc[:, h * 512:(h + 1) * 512],
                start=True, stop=True,
                skip_group_check=True,
            )

        # ---- routing iterations ----
        for r in range(3):
            # s partial -> AllReduce -> s_full
            s_loc = small_pool.tile([B, DK], F32, tag="sloc", name=f"s_loc{r}")
            nc.vector.tensor_copy(s_loc[:], s_ps[0:B, :])
            if r > 0:
                nc.vector.tensor_add(s_loc[:], s_loc[:], s_ps[B:2 * B, :])
                nc.vector.tensor_add(s_loc[:], s_loc[:], s_ps[2 * B:3 * B, :])
                nc.vector.tensor_add(s_loc[:], s_loc[:], s_ps[3 * B:4 * B, :])
            s16 = small_pool.tile([B, DK], F16, tag="s16", name=f"s16_{r}")
            nc.vector.tensor_copy(s16[:], s_loc[:])
            ar_in = dram_pool.tile([B, DK], F16, name=f"ar_in{r}")
            ar_out = dram_pool.tile([B, DK], F16, name=f"ar_out{r}")
            nc.sync.dma_start(ar_in[:], s16[:])
            nc.gpsimd.collective_compute(
                "AllReduce", ALU.add,
                replica_groups=[list(range(N_CORES))],
                ins=[ar_in.opt()],
                outs=[ar_out.opt()],
            )
            s_full = small_pool.tile([B, DK], F32, tag="sfull", name=f"s_full{r}")
            nc.sync.dma_start(s16[:], ar_out[:])
            nc.vector.tensor_copy(s_full[:], s16[:])
            if r == 0:
                nc.vector.tensor_scalar_mul(s_full[:], s_full[:], 1.0 / K)

            # squash: scale = s2/(1+s2)/sqrt(s2+eps), per (b,k); s2 = sum_d s^2
            sq = s_loc
            nc.scalar.square(sq[:], s_full[:])
            s2 = small_pool.tile([B, K], F32, tag="s2")
            nc.vector.reduce_sum(s2[:], sq.rearrange("p (d k) -> p k d", d=D),
                                 axis=mybir.AxisListType.X)
            t2 = small_pool.tile([B, K], F32, tag="t2")
            nc.vector.tensor_scalar_add(t2[:], s2[:], EPS)
            nc.scalar.sqrt(t2[:], t2[:])
            t1 = small_pool.tile([B, K], F32, tag="t1")
            nc.vector.scalar_tensor_tensor(t1[:], s2[:], 1.0, t2[:],
                                           ALU.add, ALU.mult)
            nc.vector.reciprocal(t1[:], t1[:])
            nc.vector.tensor_mul(s2[:], s2[:], t1[:])         # scale [B, K]
            o_r = s_full
            nc.vector.tensor_tensor(
                o_r.rearrange("p (d k) -> p d k", d=D),
                s_full.rearrange("p (d k) -> p d k", d=D),
                s2[:, None, :].to_broadcast([B, D, K]),
                ALU.mult,
            )

            if r == 2:
                nc.sync.dma_start(out_d, o_r[:])
                break

            # O_acc += o_r ; build O_bcast fp16 [128, (d,k)]
            if r == 0:
                nc.vector.tensor_copy(o_acc[:], o_r[:])
            else:
                nc.vector.tensor_add(o_acc[:], o_acc[:], o_r[:])
            o16 = small_pool.tile([B, DK], F16, tag="o16", name=f"o16_{r}")
            nc.vector.tensor_copy(o16[:], o_acc[:])
            o_bc = obc_pool.tile([128, DK], F16, tag="obc", name=f"obc_{r}")
            for jj in range(4):
                nc.sync.dma_start(o_bc[jj * 32:(jj + 1) * 32, :], o16[:])

            # next-iteration s accumulator
            s_ps = acc_psum.tile([128, DK], F32, tag="sacc", name=f"s{r + 1}_ps")

            # routing pass over hat chunks
            for ci in range(NCHUNK):
                gsl = slice(ci * GPC, (ci + 1) * GPC)
                hat_c = hat_sb[:, gsl, :]
                # u = sum_d hat * O_acc   (fp16 mul + pairwise tree over d)
                prod = big_pool.tile([128, GPC, DK], F16, tag="big",
                                     name=f"prod_{r}_{ci}")
                nc.vector.tensor_tensor(
                    prod[:], hat_c,
                    o_bc[:, None, :].to_broadcast([128, GPC, DK]),
                    ALU.mult,
                )
                p4 = prod.rearrange("p g (d k) -> p g d k", d=D)
                nc.vector.tensor_add(p4[:, :, 0:16, :], p4[:, :, 0:16, :],
                                     p4[:, :, 16:32, :])
                nc.vector.tensor_add(p4[:, :, 0:8, :], p4[:, :, 0:8, :],
                                     p4[:, :, 8:16, :])
                nc.vector.tensor_add(p4[:, :, 0:4, :], p4[:, :, 0:4, :],
                                     p4[:, :, 4:8, :])
                t2t = tree_pool.tile([128, GPC, 2, K], F32, tag="t2")
                nc.vector.tensor_add(t2t[:], p4[:, :, 0:2, :], p4[:, :, 2:4, :])
                u = smx_pool.tile([128, GPC, K], F32, tag="u")
                nc.vector.tensor_add(u[:], t2t[:, :, 0, :], t2t[:, :, 1, :])

                # softmax over k (free dim); u is bounded (|O|<=2), skip max-sub
                nc.scalar.activation(u[:], u[:], AF.Exp)
                z = smx_pool.tile([128, GPC], F32, tag="z")
                nc.vector.reduce_sum(z[:], u[:], axis=mybir.AxisListType.X)
                nc.vector.reciprocal(z[:], z[:])
                c16 = smx_pool.tile([128, GPC, K], F16, tag="c16")
                nc.vector.tensor_tensor(
                    c16[:], u[:], z[:, :, None].to_broadcast([128, GPC, K]),
                    ALU.mult,
                )

                # ch = c * hat ; PE partition-sum into s_ps
                ch = big_pool.tile([128, GPC, DK], F16, tag="big",
                                   name=f"ch_{r}_{ci}")
                nc.vector.tensor_tensor(
                    ch.rearrange("p g (d k) -> p g d k", d=D),
                    hat_c.rearrange("p g (d k) -> p g d k", d=D),
                    c16[:, :, None, :].to_broadcast([128, GPC, D, K]),
                    ALU.mult,
                )
                for gg in range(GPC):
                    gglob = ci * GPC + gg
                    c = gglob % 4
                    for h in (0, 1):
                        nc.tensor.matmul(
                            s_ps[32 * c:32 * (c + 1), h * 512:(h + 1) * 512],
                            lhsT=diag_sb[:],
                            rhs=ch[:, gg, h * 512:(h + 1) * 512],
                            start=(gglob < 4),
                            stop=(gglob >= NGRP - 4),
                            tile_position=(0, 32 * c),
                            skip_group_check=True,
                        )


def pack_inputs(inputs, W):
    """Host-side shard + layout pack. Returns in_maps (one dict per core)."""
    diag = np.zeros((128, B), np.float16)
    for p in range(128):
        diag[p, p % B] = 1.0

    # W: [K, J, D, I] -> per core [JL, I, D, K] fp16 -> [NPAIR, 128, DK]
    in_maps = []
    for c in range(N_CORES):
        jsl = slice(c * JL, (c + 1) * JL)
        wc = np.ascontiguousarray(
            W[:, jsl].transpose(1, 3, 2, 0), dtype=np.float16
        )  # [JL, I, D, K]
        wt = wc.reshape(NPAIR, 2 * I, DK)

        xc = inputs[:, jsl, :]  # [B, JL, I]
        xs = np.zeros((NPAIR, 128, I), np.float16)
        xt = np.ascontiguousarray(xc.transpose(1, 2, 0))  # [JL, I, B]
        xs[:, 0:I, 0:B] = xt[0::2]
        xs[:, I:128, B:2 * B] = xt[1::2]
        in_maps.append({"xs": xs, "wt": wt, "diag": diag})
    return in_maps


_CACHED_NC = None


def _install_ntff_hook():
    """Provide antenv.axon_hooks.get_axon_ntff_profile_hook when the agent
    image lacks it, by driving the injected libaxon_pjrt.so directly
    (mirrors trn_agent_boot._ntff_profile_via_ctypes)."""
    import types
    import ctypes
    import contextlib
    try:
        from antenv.axon_hooks import get_axon_ntff_profile_hook  # noqa: F401
        return True
    except ImportError:
        pass
    so_path = "/opt/axon/libaxon_pjrt.so"
    if not os.path.exists(so_path):
        return False
    lib = ctypes.CDLL(so_path)
    if not hasattr(lib, "axon_start_nrt_profile"):
        return False
    lib.axon_start_nrt_profile.argtypes = [
        ctypes.POINTER(ctypes.c_int64), ctypes.c_size_t]
    lib.axon_start_nrt_profile.restype = ctypes.c_int64
    lib.axon_stop_nrt_profile.argtypes = [ctypes.c_char_p]
    lib.axon_stop_nrt_profile.restype = ctypes.c_int64

    @contextlib.contextmanager
    def _hook(output_dir, device_ids):
        import jax
        jax.devices()
        if device_ids:
            ids = (ctypes.c_int64 * len(device_ids))(*device_ids)
            rc = lib.axon_start_nrt_profile(ids, len(device_ids))
        else:
            rc = lib.axon_start_nrt_profile(None, 0)
        if rc != 0:
            raise RuntimeError(f"axon_start_nrt_profile rc={rc}")
        try:
            yield
        finally:
            n = lib.axon_stop_nrt_profile(str(output_dir).encode())
            if n < 0:
                raise RuntimeError(f"axon_stop_nrt_profile rc={n}")

    import antenv
    mod = types.ModuleType("antenv.axon_hooks")
    mod.get_axon_ntff_profile_hook = lambda: _hook
    mod.set_axon_ntff_profile_hook = lambda h: None
    sys.modules["antenv.axon_hooks"] = mod
    antenv.axon_hooks = mod
    return True


def kernel(inputs, W):
    global _CACHED_NC
    inputs = np.asarray(inputs)
    W = np.asarray(W)
    if _CACHED_NC is None:
        _CACHED_NC = build_program()
    nc = _CACHED_NC
    in_maps = pack_inputs(inputs, W)
    trace = bool(int(os.environ.get("CAPS_TRACE", "0")))
    if trace:
        trace = _install_ntff_hook()
    res = bass_utils.run_bass_kernel_spmd(
        nc, in_maps, core_ids=list(range(N_CORES)), trace=trace,
    )
    kernel.last_results = res
    if trace and res.exec_time_ns is not None:
        print(f"HW exec time: {res.exec_time_ns} ns", file=sys.stderr)
        kernel.last_exec_time_ns = res.exec_time_ns
    out = res.results[0]["out"]  # [B, DK] fp32, identical on all cores
    return np.ascontiguousarray(
        out.reshape(B, D, K).transpose(0, 2, 1)
    ).astype(np.float32)


kernel.last_exec_time_ns = None
kernel.last_results = None



# revision 11
# speedup vs baseline: 1.0619x; 1.0619x over previous
"""Bass/Trainium2 kernel for nn_CapsuleLayer (dynamic routing capsule layer).

Reference computation:
    inputs: [B=32, J=2048, I=64], W: [K=32, J=2048, D=32, I=64]
    inputs_hat[b,k,j,d] = sum_i inputs[b,j,i] * W[k,j,d,i]
    3 routing iterations (softmax over K), output = squash(s_2)  [B, K, D]

Sharding: J (input capsules) split 8 ways -> J_loc = 256 per core.
Routing softmax (over K) is fully local; only the per-iteration
s[b,k,d] partial sums need a 64KB fp16 AllReduce.

Design notes (v2):
  - Pass A matmuls pack 2 capsules j into the full 128-partition
    contraction (block-diagonal x stations), so W streams through the
    PE exactly once at 1 column/cycle: 131072 moving rows total.
  - s0 = sum_j hat accumulated on DVE (f32) during the DMA-bound pass.
  - Routing u = sum_d O.hat: DVE computes prod = O (.) hat (fp16, 2x
    mode), the d-reduction runs on the PE as a 32-step chained
    identity-matmul PSUM accumulation (no DVE tree).
  - softmax 1/Z is folded into the per-group diagonal stationary of
    the s-reduction matmuls, eliminating the c=exp/Z multiply.
  - exp(u - 4) on the scalar engine straight out of PSUM; bias keeps
    chz = exp (.) hat inside fp16 range (validated offline).

Per-core layouts:
  xs  [128=(jp,i), NPAIR, 64]   fp16 block-diag pair stations
  wt  [128=(jp,i), NPAIR, 1024] fp16 = [(jp,i), pair, (d,k)]
  hat [128=(jj,b), 64, 32, 32]  fp16 = [(jj,b), group, d, k]
  s / o: [32, 1024] f32 = [b, (d,k)]
"""

import os
import sys
import numpy as np

import concourse.bass as bass
import concourse.mybir as mybir
import concourse.tile as tile
from concourse import bacc
from concourse import bass_utils

AF = mybir.ActivationFunctionType
ALU = mybir.AluOpType
F16 = mybir.dt.float16
F32 = mybir.dt.float32

EPS = 1e-07
N_CORES = 8
B = 32          # batch
J = 2048        # input capsules (total)
I = 64          # input capsule dim
K = 32          # output capsules
D = 32          # output capsule dim
JL = J // N_CORES          # 256 local input capsules
NPAIR = JL // 2            # 128 station pairs
NGRP = JL // 4             # 64 groups of 4 j's
DK = D * K                 # 1024
GPC = 8                    # groups per routing chunk
NCHUNK = NGRP // GPC       # 8
WG = 2                     # groups per W DMA tile (8KB descriptors)
XG = 4                     # groups per x DMA tile
EXP_BIAS = 4.0             # softmax logits bias (fp16 range, see header)


def build_program():
    """Build the SPMD bass program (same program on all 8 cores)."""
    nc = bacc.Bacc("TRN2", target_bir_lowering=False, debug=False,
                   enable_asserts=False, num_devices=N_CORES)

    xs = nc.dram_tensor("xs", [128, NPAIR, I], F16, kind="ExternalInput").ap()
    wt = nc.dram_tensor("wt", [128, NPAIR, DK], F16, kind="ExternalInput").ap()
    mask = nc.dram_tensor("mask", [128, 32], F16, kind="ExternalInput").ap()
    i128 = nc.dram_tensor("i128", [128, 128], F16, kind="ExternalInput").ap()
    repl = nc.dram_tensor("repl", [32, 128], F16, kind="ExternalInput").ap()
    out_d = nc.dram_tensor("out", [B, DK], F32, kind="ExternalOutput").ap()

    with tile.TileContext(nc) as tc:
        _emit(tc, xs, wt, mask, i128, repl, out_d)
    nc.compile()
    return nc


def _emit(tc, xs, wt, mask, i128, repl, out_d):
    nc = tc.nc
    with (
        tc.tile_pool(name="hat", bufs=1) as hat_pool,
        tc.tile_pool(name="wld", bufs=3) as w_pool,
        tc.tile_pool(name="xld", bufs=2) as x_pool,
        tc.tile_pool(name="big", bufs=2) as big_pool,
        tc.tile_pool(name="smx", bufs=2) as smx_pool,
        tc.tile_pool(name="small", bufs=1) as small_pool,
        tc.tile_pool(name="f16s", bufs=1) as f16_pool,
        tc.tile_pool(name="const", bufs=1) as const_pool,
        tc.tile_pool(name="sps", bufs=1, space="PSUM") as s_psum,
        tc.tile_pool(name="hatps", bufs=2, space="PSUM") as hat_psum,
        tc.tile_pool(name="ups", bufs=2, space="PSUM") as u_psum,
        tc.tile_pool(name="dram", bufs=6, space="DRAM") as dram_pool,
    ):
        # ---- constants ----
        mask_sb = const_pool.tile([128, 32], F16, tag="mask")
        nc.sync.dma_start(mask_sb[:], mask)
        i128_sb = const_pool.tile([128, 128], F16, tag="i128")
        nc.sync.dma_start(i128_sb[:], i128)
        repl_sb = const_pool.tile([32, 128], F16, tag="repl")
        nc.sync.dma_start(repl_sb[:], repl)
        eps_c = const_pool.tile([128, 1], F32, tag="epsc")
        nc.gpsimd.memset(eps_c[:], EPS)
        nbias_c = const_pool.tile([128, 1], F32, tag="nbias")
        nc.gpsimd.memset(nbias_c[:], -EXP_BIAS)

        # persistent hat storage: [(jj,b), group, d, k] fp16
        hat_sb = hat_pool.tile([128, NGRP, D, K], F16, tag="hat")
        s0_acc = const_pool.tile([128, DK], F16, tag="s0acc")

        # ---- Pass A: hat = x @ W (pair-packed stations); s0 on DVE ----
        xg = None
        wg_t = None
        for g in range(NGRP):
            if g % XG == 0:
                xg = x_pool.tile([128, 2 * XG, I], F16, tag="x", name=f"x{g}")
                nc.sync.dma_start(xg[:], xs[:, 2 * g:2 * g + 2 * XG, :])
            if g % WG == 0:
                wg_t = w_pool.tile([128, 2 * WG, DK], F16, tag="w",
                                   name=f"w{g}")
                nc.sync.dma_start(wg_t[:], wt[:, 2 * g:2 * g + 2 * WG, :])
            ps = hat_psum.tile([128, DK], F32, tag="hatps", name=f"hps{g}")
            for e in (0, 1):        # pair within group
                q = 2 * g + e
                qx = q % (2 * XG)
                qw = q % (2 * WG)
                for h in (0, 1):    # free-dim half (PSUM bank)
                    nc.tensor.matmul(
                        ps[64 * e:64 * e + 64, 512 * h:512 * h + 512],
                        lhsT=xg[:, qx, :],
                        rhs=wg_t[:, qw, 512 * h:512 * h + 512],
                        start=True, stop=True,
                    )
            nc.scalar.copy(hat_sb[:, g],
                           ps.rearrange("p (d k) -> p d k", d=D))
            if g == 0:
                nc.vector.tensor_copy(s0_acc[:], ps[:])
            else:
                nc.vector.tensor_add(s0_acc[:], s0_acc[:], ps[:])

        # jj-sum of s0_acc via diagonal matmuls (scale 1/K folded in place)
        nc.scalar.mul(s0_acc[:], s0_acc[:], 1.0 / K)
        s_ps = s_psum.tile([32, DK], F32, tag="sps", name="sps0")
        for h in (0, 1):
            nc.tensor.matmul(
                s_ps[:, 512 * h:512 * h + 512],
                lhsT=mask_sb[:],
                rhs=s0_acc[:, 512 * h:512 * h + 512],
                start=True, stop=True,
                skip_group_check=True,
            )

        o_acc = small_pool.tile([32, DK], F32, tag="oacc")

        # ---- routing iterations ----
        for r in range(3):
            # s partial -> AllReduce -> s_full -> squash -> o_r
            s16 = f16_pool.tile([32, DK], F16, tag="arh", name=f"s16_{r}")
            nc.vector.tensor_copy(s16[:], s_ps[:])
            ar_in = dram_pool.tile([B, DK], F16, name=f"ar_in{r}")
            ar_out = dram_pool.tile([B, DK], F16, name=f"ar_out{r}")
            nc.sync.dma_start(ar_in[:], s16[:])
            nc.gpsimd.collective_compute(
                "AllReduce", ALU.add,
                replica_groups=[list(range(N_CORES))],
                ins=[ar_in.opt()],
                outs=[ar_out.opt()],
            )
            s16b = f16_pool.tile([32, DK], F16, tag="arh", name=f"s16b_{r}")
            nc.sync.dma_start(s16b[:], ar_out[:])
            s_full = small_pool.tile([32, DK], F32, tag="sfull", name=f"sf{r}")
            nc.vector.tensor_copy(s_full[:], s16b[:])

            # squash: scale = s2/(1+s2)/sqrt(s2+eps) per (b,k); s2 = sum_d s^2
            sq = f16_pool.tile([32, DK], F16, tag="arh", name=f"sq{r}")
            nc.scalar.square(sq[:], s_full[:])
            s2 = small_pool.tile([32, K], F32, tag="s2", name=f"s2_{r}")
            nc.vector.reduce_sum(s2[:], sq.rearrange("p (d k) -> p k d", d=D),
                                 axis=mybir.AxisListType.X)
            t2 = small_pool.tile([32, K], F32, tag="t2", name=f"t2_{r}")
            nc.scalar.activation(t2[:], s2[:], AF.Sqrt, bias=eps_c[0:32, :])
            t1 = small_pool.tile([32, K], F32, tag="t1", name=f"t1_{r}")
            nc.vector.scalar_tensor_tensor(t1[:], s2[:], 1.0, t2[:],
                                           ALU.add, ALU.mult)
            nc.vector.reciprocal(t1[:], t1[:])
            nc.vector.tensor_mul(s2[:], s2[:], t1[:])       # scale [32, K]
            o_r = s_full
            nc.vector.tensor_tensor(
                o_r.rearrange("p (d k) -> p d k", d=D),
                s_full.rearrange("p (d k) -> p d k", d=D),
                s2[:, None, :].to_broadcast([32, D, K]),
                ALU.mult,
            )

            if r == 2:
                nc.sync.dma_start(out_d, o_r[:])
                break

            # O_acc accumulation; broadcast to 128 partitions via PE
            if r == 0:
                nc.vector.tensor_copy(o_acc[:], o_r[:])
            else:
                nc.vector.tensor_add(o_acc[:], o_acc[:], o_r[:])
            o16 = f16_pool.tile([32, DK], F16, tag="o16", name=f"o16_{r}")
            nc.vector.tensor_copy(o16[:], o_acc[:])
            obps = hat_psum.tile([128, DK], F32, tag="hatps", name=f"obps{r}")
            for h in (0, 1):
                nc.tensor.matmul(
                    obps[:, 512 * h:512 * h + 512],
                    lhsT=repl_sb[:],
                    rhs=o16[:, 512 * h:512 * h + 512],
                    start=True, stop=True,
                    skip_group_check=True,
                )
            o_bc = const_pool.tile([128, DK], F16, tag="obc", name=f"obc{r}")
            nc.scalar.copy(o_bc[:], obps[:])
            o_bc4 = o_bc.rearrange("p (d k) -> p d k", d=D)

            # next-iteration s accumulator
            s_ps = s_psum.tile([32, DK], F32, tag="sps", name=f"sps{r + 1}")

            # routing pass over hat chunks
            for ci in range(NCHUNK):
                hat_c = hat_sb[:, ci * GPC:(ci + 1) * GPC]   # [128,8,32,32]
                # prod = hat (.) O_acc (fp16, 2x mode); stored d-major so
                # each d-slice is a contiguous 2D moving AP for the PE
                prod = big_pool.tile([128, D, GPC, K], F16, tag="big",
                                     name=f"pr{r}_{ci}")
                nc.vector.tensor_tensor(
                    prod.rearrange("p d g k -> p g d k"), hat_c,
                    o_bc4[:, None, :, :].to_broadcast([128, GPC, D, K]),
                    ALU.mult,
                )
                # u = sum_d prod : chained identity-matmul PSUM accumulation
                u_ps = u_psum.tile([128, GPC, K], F32, tag="ups",
                                   name=f"u{r}_{ci}")
                for dd in range(D):
                    nc.tensor.matmul(
                        u_ps[:],
                        lhsT=i128_sb[:],
                        rhs=prod[:, dd],
                        start=(dd == 0), stop=(dd == D - 1),
                        skip_group_check=True,
                    )
                # softmax over k (free dim); exp(u - bias) out of PSUM
                exp16 = smx_pool.tile([128, GPC, K], F16, tag="e16",
                                      name=f"e{r}_{ci}")
                nc.scalar.activation(exp16[:], u_ps[:], AF.Exp,
                                     bias=nbias_c[:])
                zt = smx_pool.tile([128, GPC], F32, tag="z", name=f"z{r}_{ci}")
                nc.vector.reduce_sum(zt[:], exp16[:],
                                     axis=mybir.AxisListType.X)
                nc.vector.reciprocal(zt[:], zt[:])
                # zdiag[p, g, m] = (1/Z)[p, g] * (m == p%32)
                zdiag = smx_pool.tile([128, GPC, 32], F16, tag="zd",
                                      name=f"zd{r}_{ci}")
                nc.vector.tensor_tensor(
                    zdiag[:],
                    mask_sb[:, None, :].to_broadcast([128, GPC, 32]),
                    zt[:, :, None].to_broadcast([128, GPC, 32]),
                    ALU.mult,
                )
                # chz = exp (.) hat ; 1/Z applied by the PE stationary
                chz = big_pool.tile([128, GPC, D, K], F16, tag="big",
                                    name=f"ch{r}_{ci}")
                nc.vector.tensor_tensor(
                    chz[:], hat_c,
                    exp16[:, :, None, :].to_broadcast([128, GPC, D, K]),
                    ALU.mult,
                )
                for gg in range(GPC):
                    gglob = ci * GPC + gg
                    for h in (0, 1):
                        nc.tensor.matmul(
                            s_ps[:, 512 * h:512 * h + 512],
                            lhsT=zdiag[:, gg, :],
                            rhs=chz[:, gg, 16 * h:16 * h + 16, :],
                            start=(gglob == 0), stop=(gglob == NGRP - 1),
                            skip_group_check=True,
                        )


def pack_inputs(inputs, W):
    """Host-side shard + layout pack. Returns in_maps (one dict per core)."""
    mask = np.zeros((128, 32), np.float16)
    mask[np.arange(128), np.arange(128) % 32] = 1.0
    i128 = np.eye(128, dtype=np.float16)
    repl = np.zeros((32, 128), np.float16)
    repl[np.arange(128) % 32, np.arange(128)] = 1.0

    in_maps = []
    for c in range(N_CORES):
        jsl = slice(c * JL, (c + 1) * JL)
        xc = inputs[:, jsl, :]                       # [B, JL, I]
        xt = np.ascontiguousarray(xc.transpose(2, 1, 0), dtype=np.float16)
        # xs station: [p=(jp,i), pair, m=(jp,b)] block-diagonal
        xs = np.zeros((128, NPAIR, I), np.float16)
        xs[0:I, :, 0:B] = xt[:, 0::2, :]             # jp=0: j = 2q
        xs[I:128, :, B:2 * B] = xt[:, 1::2, :]       # jp=1: j = 2q+1
        # W: [K, J, D, I] -> [JL, I, DK] -> [p=(jp,i), pair, dk]
        wc = np.ascontiguousarray(
            W[:, jsl].transpose(1, 3, 2, 0), dtype=np.float16
        ).reshape(JL, I, DK)
        wtp = np.empty((128, NPAIR, DK), np.float16)
        wtp[0:I] = wc[0::2].transpose(1, 0, 2)
        wtp[I:128] = wc[1::2].transpose(1, 0, 2)
        in_maps.append({"xs": xs, "wt": wtp, "mask": mask,
                        "i128": i128, "repl": repl})
    return in_maps


_CACHED_NC = None


def _install_ntff_hook():
    """Provide antenv.axon_hooks.get_axon_ntff_profile_hook when the agent
    image lacks it, by driving the injected libaxon_pjrt.so directly
    (mirrors trn_agent_boot._ntff_profile_via_ctypes)."""
    import types
    import ctypes
    import contextlib
    try:
        from antenv.axon_hooks import get_axon_ntff_profile_hook  # noqa: F401
        return True
    except ImportError:
        pass
    so_path = "/opt/axon/libaxon_pjrt.so"
    if not os.path.exists(so_path):
        return False
    lib = ctypes.CDLL(so_path)
    if not hasattr(lib, "axon_start_nrt_profile"):
        return False
    lib.axon_start_nrt_profile.argtypes = [
        ctypes.POINTER(ctypes.c_int64), ctypes.c_size_t]
    lib.axon_start_nrt_profile.restype = ctypes.c_int64
    lib.axon_stop_nrt_profile.argtypes = [ctypes.c_char_p]
    lib.axon_stop_nrt_profile.restype = ctypes.c_int64

    @contextlib.contextmanager
    def _hook(output_dir, device_ids):
        import jax
        jax.devices()
        if device_ids:
            ids = (ctypes.c_int64 * len(device_ids))(*device_ids)
            rc = lib.axon_start_nrt_profile(ids, len(device_ids))
        else:
            rc = lib.axon_start_nrt_profile(None, 0)
        if rc != 0:
            raise RuntimeError(f"axon_start_nrt_profile rc={rc}")
        try:
            yield
        finally:
            n = lib.axon_stop_nrt_profile(str(output_dir).encode())
            if n < 0:
                raise RuntimeError(f"axon_stop_nrt_profile rc={n}")

    import antenv
    mod = types.ModuleType("antenv.axon_hooks")
    mod.get_axon_ntff_profile_hook = lambda: _hook
    mod.set_axon_ntff_profile_hook = lambda h: None
    sys.modules["antenv.axon_hooks"] = mod
    antenv.axon_hooks = mod
    return True


def kernel(inputs, W):
    global _CACHED_NC
    inputs = np.asarray(inputs)
    W = np.asarray(W)
    if _CACHED_NC is None:
        _CACHED_NC = build_program()
    nc = _CACHED_NC
    in_maps = pack_inputs(inputs, W)
    trace = bool(int(os.environ.get("CAPS_TRACE", "0")))
    if trace:
        trace = _install_ntff_hook()
    res = bass_utils.run_bass_kernel_spmd(
        nc, in_maps, core_ids=list(range(N_CORES)), trace=trace,
    )
    kernel.last_results = res
    if trace and res.exec_time_ns is not None:
        print(f"HW exec time: {res.exec_time_ns} ns", file=sys.stderr)
        kernel.last_exec_time_ns = res.exec_time_ns
    out = res.results[0]["out"]  # [B, DK] fp32, identical on all cores
    return np.ascontiguousarray(
        out.reshape(B, D, K).transpose(0, 2, 1)
    ).astype(np.float32)


kernel.last_exec_time_ns = None
kernel.last_results = None


# revision 13
# speedup vs baseline: 1.3721x; 1.2921x over previous
"""Bass/Trainium2 kernel for nn_CapsuleLayer (dynamic routing capsule layer).

Reference computation:
    inputs: [B=32, J=2048, I=64], W: [K=32, J=2048, D=32, I=64]
    inputs_hat[b,k,j,d] = sum_i inputs[b,j,i] * W[k,j,d,i]
    3 routing iterations (softmax over K), output = squash(s_2)  [B, K, D]

Sharding: J (input capsules) split 8 ways -> J_loc = 256 per core.
Routing softmax (over K) is fully local; only the per-iteration
s[b,k,d] partial sums need a 64KB fp16 AllReduce.

Design notes (v2):
  - Pass A matmuls pack 2 capsules j into the full 128-partition
    contraction (block-diagonal x stations), so W streams through the
    PE exactly once at 1 column/cycle: 131072 moving rows total.
  - s0 = sum_j hat accumulated on DVE (f32) during the DMA-bound pass.
  - Routing u = sum_d O.hat: DVE computes prod = O (.) hat (fp16, 2x
    mode), the d-reduction runs on the PE as a 32-step chained
    identity-matmul PSUM accumulation (no DVE tree).
  - softmax 1/Z is folded into the per-group diagonal stationary of
    the s-reduction matmuls, eliminating the c=exp/Z multiply.
  - exp(u - 4) on the scalar engine straight out of PSUM; bias keeps
    chz = exp (.) hat inside fp16 range (validated offline).

Per-core layouts:
  xs  [128=(jp,i), NPAIR, 64]   fp16 block-diag pair stations
  wt  [128=(jp,i), NPAIR, 1024] fp16 = [(jp,i), pair, (d,k)]
  hat [128=(jj,b), 64, 32, 32]  fp16 = [(jj,b), group, d, k]
  s / o: [32, 1024] f32 = [b, (d,k)]
"""

import os
import sys
import numpy as np

import concourse.bass as bass
import concourse.mybir as mybir
import concourse.tile as tile
from concourse import bacc
from concourse import bass_utils

AF = mybir.ActivationFunctionType
ALU = mybir.AluOpType
F16 = mybir.dt.float16
F32 = mybir.dt.float32

EPS = 1e-07
N_CORES = 8
B = 32          # batch
J = 2048        # input capsules (total)
I = 64          # input capsule dim
K = 32          # output capsules
D = 32          # output capsule dim
JL = J // N_CORES          # 256 local input capsules
NPAIR = JL // 2            # 128 station pairs
NGRP = JL // 4             # 64 groups of 4 j's
DK = D * K                 # 1024
GPC = 8                    # groups per routing chunk
NCHUNK = NGRP // GPC       # 8
WG = 2                     # groups per W DMA tile (8KB descriptors)
XG = 4                     # groups per x DMA tile
EXP_BIAS = 4.0             # softmax logits bias (fp16 range, see header)


def build_program():
    """Build the SPMD bass program (same program on all 8 cores)."""
    nc = bacc.Bacc("TRN2", target_bir_lowering=False, debug=False,
                   enable_asserts=False, num_devices=N_CORES)

    xs = nc.dram_tensor("xs", [128, NPAIR, I], F16, kind="ExternalInput").ap()
    wt = nc.dram_tensor("wt", [128, NPAIR, DK], F16, kind="ExternalInput").ap()
    mask = nc.dram_tensor("mask", [128, 32], F16, kind="ExternalInput").ap()
    i128 = nc.dram_tensor("i128", [128, 128], F16, kind="ExternalInput").ap()
    repl = nc.dram_tensor("repl", [32, 128], F16, kind="ExternalInput").ap()
    out_d = nc.dram_tensor("out", [B, DK], F32, kind="ExternalOutput").ap()

    with tile.TileContext(nc) as tc:
        _emit(tc, xs, wt, mask, i128, repl, out_d)
    nc.compile()
    return nc


def _emit(tc, xs, wt, mask, i128, repl, out_d):
    nc = tc.nc
    with (
        tc.tile_pool(name="hat", bufs=1) as hat_pool,
        tc.tile_pool(name="wld", bufs=3) as w_pool,
        tc.tile_pool(name="xld", bufs=2) as x_pool,
        tc.tile_pool(name="big", bufs=2) as big_pool,
        tc.tile_pool(name="smx", bufs=2) as smx_pool,
        tc.tile_pool(name="small", bufs=1) as small_pool,
        tc.tile_pool(name="f16s", bufs=1) as f16_pool,
        tc.tile_pool(name="const", bufs=1) as const_pool,
        tc.tile_pool(name="sps", bufs=1, space="PSUM") as s_psum,
        tc.tile_pool(name="hatps", bufs=2, space="PSUM") as hat_psum,
        tc.tile_pool(name="ups", bufs=2, space="PSUM") as u_psum,
        tc.tile_pool(name="dram", bufs=6, space="DRAM") as dram_pool,
    ):
        # ---- constants ----
        mask_sb = const_pool.tile([128, 32], F16, tag="mask")
        nc.sync.dma_start(mask_sb[:], mask)
        i128_sb = const_pool.tile([128, 128], F16, tag="i128")
        nc.sync.dma_start(i128_sb[:], i128)
        repl_sb = const_pool.tile([32, 128], F16, tag="repl")
        nc.sync.dma_start(repl_sb[:], repl)
        eps_c = const_pool.tile([128, 1], F32, tag="epsc")
        nc.gpsimd.memset(eps_c[:], EPS)
        nbias_c = const_pool.tile([128, 1], F32, tag="nbias")
        nc.gpsimd.memset(nbias_c[:], -EXP_BIAS)

        # persistent hat storage: [(jj,b), group, d, k] fp16
        hat_sb = hat_pool.tile([128, NGRP, D, K], F16, tag="hat")
        s0_acc = const_pool.tile([128, DK], F16, tag="s0acc")

        # ---- Pass A: hat = x @ W (pair-packed stations); s0 on DVE ----
        xg = None
        wg_t = None
        for g in range(NGRP):
            if g % XG == 0:
                xg = x_pool.tile([128, 2 * XG, I], F16, tag="x", name=f"x{g}")
                nc.sync.dma_start(xg[:], xs[:, 2 * g:2 * g + 2 * XG, :])
            if g % WG == 0:
                wg_t = w_pool.tile([128, 2 * WG, DK], F16, tag="w",
                                   name=f"w{g}")
                nc.sync.dma_start(wg_t[:], wt[:, 2 * g:2 * g + 2 * WG, :])
            ps = hat_psum.tile([128, DK], F32, tag="hatps", name=f"hps{g}")
            for e in (0, 1):        # pair within group
                q = 2 * g + e
                qx = q % (2 * XG)
                qw = q % (2 * WG)
                for h in (0, 1):    # free-dim half (PSUM bank)
                    nc.tensor.matmul(
                        ps[64 * e:64 * e + 64, 512 * h:512 * h + 512],
                        lhsT=xg[:, qx, :],
                        rhs=wg_t[:, qw, 512 * h:512 * h + 512],
                        start=True, stop=True,
                    )
            nc.scalar.copy(hat_sb[:, g],
                           ps.rearrange("p (d k) -> p d k", d=D))
            if g == 0:
                nc.vector.tensor_copy(s0_acc[:], ps[:])
            else:
                nc.vector.tensor_add(s0_acc[:], s0_acc[:], ps[:])

        # jj-sum of s0_acc via diagonal matmuls (scale 1/K folded in place)
        nc.scalar.mul(s0_acc[:], s0_acc[:], 1.0 / K)
        s_ps = s_psum.tile([32, DK], F32, tag="sps", name="sps0")
        for h in (0, 1):
            nc.tensor.matmul(
                s_ps[:, 512 * h:512 * h + 512],
                lhsT=mask_sb[:],
                rhs=s0_acc[:, 512 * h:512 * h + 512],
                start=True, stop=True,
                skip_group_check=True,
            )

        o_acc = small_pool.tile([32, DK], F32, tag="oacc")

        # ---- routing iterations ----
        for r in range(3):
            # s partial -> AllReduce -> s_full -> squash -> o_r
            s16 = f16_pool.tile([32, DK], F16, tag="arh", name=f"s16_{r}")
            nc.vector.tensor_copy(s16[:], s_ps[:])
            ar_in = dram_pool.tile([B, DK], F16, name=f"ar_in{r}")
            ar_out = dram_pool.tile([B, DK], F16, name=f"ar_out{r}")
            nc.sync.dma_start(ar_in[:], s16[:])
            nc.gpsimd.collective_compute(
                "AllReduce", ALU.add,
                replica_groups=[list(range(N_CORES))],
                ins=[ar_in.opt()],
                outs=[ar_out.opt()],
            )
            s16b = f16_pool.tile([32, DK], F16, tag="arh", name=f"s16b_{r}")
            nc.sync.dma_start(s16b[:], ar_out[:])
            s_full = small_pool.tile([32, DK], F32, tag="sfull", name=f"sf{r}")
            nc.vector.tensor_copy(s_full[:], s16b[:])

            # squash: scale = s2/(1+s2)/sqrt(s2+eps) per (b,k); s2 = sum_d s^2
            sq = f16_pool.tile([32, DK], F16, tag="arh", name=f"sq{r}")
            nc.scalar.square(sq[:], s_full[:])
            s2 = small_pool.tile([32, K], F32, tag="s2", name=f"s2_{r}")
            nc.vector.reduce_sum(s2[:], sq.rearrange("p (d k) -> p k d", d=D),
                                 axis=mybir.AxisListType.X)
            t2 = small_pool.tile([32, K], F32, tag="t2", name=f"t2_{r}")
            nc.scalar.activation(t2[:], s2[:], AF.Sqrt, bias=eps_c[0:32, :])
            t1 = small_pool.tile([32, K], F32, tag="t1", name=f"t1_{r}")
            nc.vector.scalar_tensor_tensor(t1[:], s2[:], 1.0, t2[:],
                                           ALU.add, ALU.mult)
            nc.vector.reciprocal(t1[:], t1[:])
            nc.vector.tensor_mul(s2[:], s2[:], t1[:])       # scale [32, K]
            o_r = s_full
            nc.vector.tensor_tensor(
                o_r.rearrange("p (d k) -> p d k", d=D),
                s_full.rearrange("p (d k) -> p d k", d=D),
                s2[:, None, :].to_broadcast([32, D, K]),
                ALU.mult,
            )

            if r == 2:
                nc.sync.dma_start(out_d, o_r[:])
                break

            # O_acc accumulation; broadcast to 128 partitions via PE
            if r == 0:
                nc.vector.tensor_copy(o_acc[:], o_r[:])
            else:
                nc.vector.tensor_add(o_acc[:], o_acc[:], o_r[:])
            o16 = f16_pool.tile([32, DK], F16, tag="o16", name=f"o16_{r}")
            nc.vector.tensor_copy(o16[:], o_acc[:])
            obps = hat_psum.tile([128, DK], F32, tag="hatps", name=f"obps{r}")
            for h in (0, 1):
                nc.tensor.matmul(
                    obps[:, 512 * h:512 * h + 512],
                    lhsT=repl_sb[:],
                    rhs=o16[:, 512 * h:512 * h + 512],
                    start=True, stop=True,
                    skip_group_check=True,
                )
            o_bc = const_pool.tile([128, DK], F16, tag="obc", name=f"obc{r}")
            nc.scalar.copy(o_bc[:], obps[:])
            o_bc4 = o_bc.rearrange("p (d k) -> p d k", d=D)

            # next-iteration s accumulator
            s_ps = s_psum.tile([32, DK], F32, tag="sps", name=f"sps{r + 1}")

            # routing pass over hat chunks, software-pipelined so the
            # prod-mul of chunk c+1 (DVE) overlaps the u-matmuls of chunk
            # c (PE) and the chz/s phase of chunk c-1.
            def phase1(ci):
                hat_c = hat_sb[:, ci * GPC:(ci + 1) * GPC]   # [128,8,32,32]
                # prod = hat (.) O_acc (fp16, 2x mode); stored d-major so
                # each d-slice is a contiguous 2D moving AP for the PE
                prod = big_pool.tile([128, D, GPC, K], F16, tag="big",
                                     name=f"pr{r}_{ci}")
                nc.vector.tensor_tensor(
                    prod.rearrange("p d g k -> p g d k"), hat_c,
                    o_bc4[:, None, :, :].to_broadcast([128, GPC, D, K]),
                    ALU.mult,
                )
                # u = sum_d prod : chained identity-matmul PSUM accumulation
                u_ps = u_psum.tile([128, GPC, K], F32, tag="ups",
                                   name=f"u{r}_{ci}")
                for dd in range(D):
                    nc.tensor.matmul(
                        u_ps[:],
                        lhsT=i128_sb[:],
                        rhs=prod[:, dd],
                        start=(dd == 0), stop=(dd == D - 1),
                        skip_group_check=True,
                    )
                return u_ps

            def phase2(ci, u_ps):
                hat_c = hat_sb[:, ci * GPC:(ci + 1) * GPC]
                # softmax over k (free dim); exp(u - bias) out of PSUM
                exp16 = smx_pool.tile([128, GPC, K], F16, tag="e16",
                                      name=f"e{r}_{ci}")
                nc.scalar.activation(exp16[:], u_ps[:], AF.Exp,
                                     bias=nbias_c[:])
                zt = smx_pool.tile([128, GPC], F32, tag="z", name=f"z{r}_{ci}")
                nc.vector.reduce_sum(zt[:], exp16[:],
                                     axis=mybir.AxisListType.X)
                nc.vector.reciprocal(zt[:], zt[:])
                # zdiag[p, g, m] = (1/Z)[p, g] * (m == p%32)
                zdiag = smx_pool.tile([128, GPC, 32], F16, tag="zd",
                                      name=f"zd{r}_{ci}")
                nc.vector.tensor_tensor(
                    zdiag[:],
                    mask_sb[:, None, :].to_broadcast([128, GPC, 32]),
                    zt[:, :, None].to_broadcast([128, GPC, 32]),
                    ALU.mult,
                )
                # chz = exp (.) hat ; 1/Z applied by the PE stationary
                chz = big_pool.tile([128, GPC, D, K], F16, tag="big",
                                    name=f"ch{r}_{ci}")
                nc.vector.tensor_tensor(
                    chz[:], hat_c,
                    exp16[:, :, None, :].to_broadcast([128, GPC, D, K]),
                    ALU.mult,
                )
                for gg in range(GPC):
                    gglob = ci * GPC + gg
                    for h in (0, 1):
                        nc.tensor.matmul(
                            s_ps[:, 512 * h:512 * h + 512],
                            lhsT=zdiag[:, gg, :],
                            rhs=chz[:, gg, 16 * h:16 * h + 16, :],
                            start=(gglob == 0), stop=(gglob == NGRP - 1),
                            skip_group_check=True,
                        )

            u_prev = None
            for ci in range(NCHUNK):
                u_cur = phase1(ci)
                if u_prev is not None:
                    phase2(ci - 1, u_prev)
                u_prev = u_cur
            phase2(NCHUNK - 1, u_prev)


def pack_inputs(inputs, W):
    """Host-side shard + layout pack. Returns in_maps (one dict per core)."""
    mask = np.zeros((128, 32), np.float16)
    mask[np.arange(128), np.arange(128) % 32] = 1.0
    i128 = np.eye(128, dtype=np.float16)
    repl = np.zeros((32, 128), np.float16)
    repl[np.arange(128) % 32, np.arange(128)] = 1.0

    in_maps = []
    for c in range(N_CORES):
        jsl = slice(c * JL, (c + 1) * JL)
        xc = inputs[:, jsl, :]                       # [B, JL, I]
        xt = np.ascontiguousarray(xc.transpose(2, 1, 0), dtype=np.float16)
        # xs station: [p=(jp,i), pair, m=(jp,b)] block-diagonal
        xs = np.zeros((128, NPAIR, I), np.float16)
        xs[0:I, :, 0:B] = xt[:, 0::2, :]             # jp=0: j = 2q
        xs[I:128, :, B:2 * B] = xt[:, 1::2, :]       # jp=1: j = 2q+1
        # W: [K, J, D, I] -> [JL, I, DK] -> [p=(jp,i), pair, dk]
        wc = np.ascontiguousarray(
            W[:, jsl].transpose(1, 3, 2, 0), dtype=np.float16
        ).reshape(JL, I, DK)
        wtp = np.empty((128, NPAIR, DK), np.float16)
        wtp[0:I] = wc[0::2].transpose(1, 0, 2)
        wtp[I:128] = wc[1::2].transpose(1, 0, 2)
        in_maps.append({"xs": xs, "wt": wtp, "mask": mask,
                        "i128": i128, "repl": repl})
    return in_maps


_CACHED_NC = None


def _install_ntff_hook():
    """Provide antenv.axon_hooks.get_axon_ntff_profile_hook when the agent
    image lacks it, by driving the injected libaxon_pjrt.so directly
    (mirrors trn_agent_boot._ntff_profile_via_ctypes)."""
    import types
    import ctypes
    import contextlib
    try:
        from antenv.axon_hooks import get_axon_ntff_profile_hook  # noqa: F401
        return True
    except ImportError:
        pass
    so_path = "/opt/axon/libaxon_pjrt.so"
    if not os.path.exists(so_path):
        return False
    lib = ctypes.CDLL(so_path)
    if not hasattr(lib, "axon_start_nrt_profile"):
        return False
    lib.axon_start_nrt_profile.argtypes = [
        ctypes.POINTER(ctypes.c_int64), ctypes.c_size_t]
    lib.axon_start_nrt_profile.restype = ctypes.c_int64
    lib.axon_stop_nrt_profile.argtypes = [ctypes.c_char_p]
    lib.axon_stop_nrt_profile.restype = ctypes.c_int64

    @contextlib.contextmanager
    def _hook(output_dir, device_ids):
        import jax
        jax.devices()
        if device_ids:
            ids = (ctypes.c_int64 * len(device_ids))(*device_ids)
            rc = lib.axon_start_nrt_profile(ids, len(device_ids))
        else:
            rc = lib.axon_start_nrt_profile(None, 0)
        if rc != 0:
            raise RuntimeError(f"axon_start_nrt_profile rc={rc}")
        try:
            yield
        finally:
            n = lib.axon_stop_nrt_profile(str(output_dir).encode())
            if n < 0:
                raise RuntimeError(f"axon_stop_nrt_profile rc={n}")

    import antenv
    mod = types.ModuleType("antenv.axon_hooks")
    mod.get_axon_ntff_profile_hook = lambda: _hook
    mod.set_axon_ntff_profile_hook = lambda h: None
    sys.modules["antenv.axon_hooks"] = mod
    antenv.axon_hooks = mod
    return True


def kernel(inputs, W):
    global _CACHED_NC
    inputs = np.asarray(inputs)
    W = np.asarray(W)
    if _CACHED_NC is None:
        _CACHED_NC = build_program()
    nc = _CACHED_NC
    in_maps = pack_inputs(inputs, W)
    trace = bool(int(os.environ.get("CAPS_TRACE", "0")))
    if trace:
        trace = _install_ntff_hook()
    res = bass_utils.run_bass_kernel_spmd(
        nc, in_maps, core_ids=list(range(N_CORES)), trace=trace,
    )
    kernel.last_results = res
    if trace and res.exec_time_ns is not None:
        print(f"HW exec time: {res.exec_time_ns} ns", file=sys.stderr)
        kernel.last_exec_time_ns = res.exec_time_ns
    out = res.results[0]["out"]  # [B, DK] fp32, identical on all cores
    return np.ascontiguousarray(
        out.reshape(B, D, K).transpose(0, 2, 1)
    ).astype(np.float32)


kernel.last_exec_time_ns = None
kernel.last_results = None
